# revision 9
# baseline (speedup 1.0000x reference)
"""Trainium2 Bass kernel for nn_CrossAttention (XCA-style channel attention).

Full-input contract: kernel(**inputs) takes the complete tensors, shards the
batch over 8 NeuronCores (2 batches per core, pure data parallel, no
collectives), runs one SPMD Bass program, and reassembles the full output.

Per-core math (per batch), all fp32 with float32r matmuls:
  kT[n,o] = sum_c x[c,n]   wkT[c,o]      (transposed layout: n on partitions)
  qT[n,o] = sum_c qry[c,n] wqT[c,o]
  v[o,n]  = sum_c wvT[c,o] x[c,n]        (natural layout)
  S^T[d,c] = sum_n kT[n,d] qT[n,c]       (full-cross; only head-diag blocks used)
  norms: |k_d|^2, |q_c|^2 via DVE squares + ones-matmul partition reduction
  expST = maskA * exp(S^T * (1/|k_d|) * (temp[h(c)]/|q_c|))   (zeros off-diag)
  denom[c] = sum_d expST[d,c]            (via ones-column appended to v)
  av[c,n] = (1/denom[c]) * sum_d expST[d,c] v[d,n]
  out[o,n] = sum_c wpT[c,o] av[c,n]
Softmax needs no max-subtraction: scores are cosine similarities * temperature.
"""

import os
import sys

import numpy as np

for p in ("/opt/trn_rl_repo", os.path.expanduser("~/.axon_site/_ro/trn_rl_repo")):
    if os.path.isdir(p) and p not in sys.path:
        sys.path.insert(0, p)

import concourse.bass as bass
from concourse import bacc
import concourse.mybir as mybir
import concourse.tile as tile
from concourse.alu_op_type import AluOpType
from concourse.bass_utils import run_bass_kernel_spmd

NUM_HEADS = 8
B, C, QC, H, W = 16, 384, 192, 64, 64
NTOT = H * W          # 4096 spatial positions
DC = C // NUM_HEADS   # 48 channels per head
NCORES = 8
BLOC = B // NCORES    # 2 batches per core
VPAD = 8              # extra ones-columns appended to v for the denominator

F32 = mybir.dt.float32
F32R = mybir.dt.float32r
AF = mybir.ActivationFunctionType

# chunk-pair (d-chunk k, c-chunk m) blocks of the 128-partition-tiled [384,384]
# head-block-diagonal matrix that are not identically zero (heads are 48 wide)
NZ_BLOCKS = {0: [0, 1], 1: [0, 1, 2], 2: [1, 2]}


def _f32r(ap):
    return ap.bitcast(F32R)


def build_bass(ntot=NTOT, bloc=BLOC, finalize=True):
    nc = bacc.Bacc(None)
    nsup = ntot // 512
    nch_per_sup = 4  # 128-chunks per 512 superchunk
    nch = nsup * nch_per_sup

    x_d = nc.declare_dram_parameter("x", [bloc, C, ntot], F32, isOutput=False)
    qry_d = nc.declare_dram_parameter("query", [bloc, QC, ntot], F32, isOutput=False)
    wk_d = nc.declare_dram_parameter("wkT", [C, C], F32, isOutput=False)
    wv_d = nc.declare_dram_parameter("wvT", [C, C], F32, isOutput=False)
    wq_d = nc.declare_dram_parameter("wqT", [QC, C], F32, isOutput=False)
    wp_d = nc.declare_dram_parameter("wpT", [C, C], F32, isOutput=False)
    ma_d = nc.declare_dram_parameter("maskA", [C, C], F32, isOutput=False)
    tr_d = nc.declare_dram_parameter("trow", [1, C], F32, isOutput=False)
    out_d = nc.declare_dram_parameter("out", [bloc, C, ntot], F32, isOutput=True)

    with tile.TileContext(nc) as tc:
        with (
            tc.tile_pool(name="weights", bufs=1) as wpool,
            tc.tile_pool(name="xq", bufs=2) as xpool,
            tc.tile_pool(name="ktq", bufs=3) as ktqp,
            tc.tile_pool(name="sq", bufs=2) as sqp,
            tc.tile_pool(name="v", bufs=1) as vpool,
            tc.tile_pool(name="expst", bufs=1) as epool,
            tc.tile_pool(name="small", bufs=2) as spool,
            tc.tile_pool(name="avs", bufs=2) as avsp,
            tc.tile_pool(name="outs", bufs=3) as outp,
            tc.tile_pool(name="psS", bufs=1, space="PSUM") as ps_S,
            tc.tile_pool(name="psN", bufs=1, space="PSUM") as ps_N,
            tc.tile_pool(name="psW", bufs=3, space="PSUM") as ps_W,
        ):
            # ---- constants ----
            wk = wpool.tile([128, 3, C], F32R, tag="wk")
            wv = wpool.tile([128, 3, C], F32R, tag="wv")
            wp = wpool.tile([128, 3, C], F32R, tag="wp")
            ma = wpool.tile([128, 3, C], F32, tag="ma")
            for kc in range(3):
                nc.sync.dma_start(out=wk[:, kc, :], in_=_f32r(wk_d[128 * kc:128 * (kc + 1), :]))
                nc.sync.dma_start(out=wv[:, kc, :], in_=_f32r(wv_d[128 * kc:128 * (kc + 1), :]))
                nc.sync.dma_start(out=wp[:, kc, :], in_=_f32r(wp_d[128 * kc:128 * (kc + 1), :]))
                nc.sync.dma_start(out=ma[:, kc, :], in_=ma_d[128 * kc:128 * (kc + 1), :])
            wq1 = wpool.tile([128, C], F32R, tag="wq1")
            wq2 = wpool.tile([64, C], F32R, tag="wq2")
            nc.sync.dma_start(out=wq1, in_=_f32r(wq_d[0:128, :]))
            nc.sync.dma_start(out=wq2, in_=_f32r(wq_d[128:192, :]))
            trow = wpool.tile([1, C], F32, tag="trow")
            nc.sync.dma_start(out=trow, in_=tr_d[:, :])
            onesf = wpool.tile([128, 128], F32, tag="onesf")
            nc.vector.memset(onesf, 1.0)
            ones = wpool.tile([128, 128], F32R, tag="ones")
            nc.vector.tensor_copy(out=ones, in_=onesf)

            for b in range(bloc):
                # ---- per-batch persistent tiles ----
                v_sb = [vpool.tile([128, ntot + VPAD], F32R, tag=f"v{k}", name=f"v{k}") for k in range(3)]
                for k in range(3):
                    nc.vector.tensor_copy(
                        out=v_sb[k][:, ntot:ntot + VPAD], in_=onesf[:, 0:VPAD])
                s_ps = [ps_S.tile([128, C], F32, tag=f"S{m}", name=f"S{m}") for m in range(3)]
                kn2_ps = ps_N.tile([1, C], F32, tag="kn2")
                qn2_ps = ps_N.tile([1, C], F32, tag="qn2")

                # ================= phase 1: kT, qT, v, S^T, norm sums ========
                for ns in range(nsup):
                    nsl5 = slice(ns * 512, (ns + 1) * 512)
                    xs = xpool.tile([128, 3, 512], F32R, tag="xs")
                    for kc in range(3):
                        nc.sync.dma_start(
                            out=xs[:, kc, :], in_=_f32r(x_d[b, 128 * kc:128 * (kc + 1), nsl5]))
                    qy1 = xpool.tile([128, 512], F32R, tag="qy1")
                    qy2 = xpool.tile([64, 512], F32R, tag="qy2")
                    nc.sync.dma_start(out=qy1, in_=_f32r(qry_d[b, 0:128, nsl5]))
                    nc.sync.dma_start(out=qy2, in_=_f32r(qry_d[b, 128:192, nsl5]))

                    for nn in range(nch_per_sup):
                        t = ns * nch_per_sup + nn
                        nsl1 = slice(nn * 128, (nn + 1) * 128)
                        # kT chunk: [n128, o384]
                        kp = ps_W.tile([128, C], F32, tag="w")
                        for kc in range(3):
                            nc.tensor.matmul(
                                kp, lhsT=_f32r(xs[:, kc, nsl1]), rhs=_f32r(wk[:, kc, :]),
                                start=(kc == 0), stop=(kc == 2))
                        kt_sb = ktqp.tile([128, C], F32R, tag="kt")
                        nc.vector.tensor_copy(out=kt_sb, in_=kp)
                        ksq = sqp.tile([128, C], F32R, tag="ksq")
                        nc.vector.tensor_mul(ksq, kt_sb.bitcast(F32), kt_sb.bitcast(F32))
                        nc.tensor.matmul(
                            kn2_ps, lhsT=_f32r(ones[:, 0:1]), rhs=_f32r(ksq),
                            start=(t == 0), stop=(t == nch - 1))
                        # qT chunk: [n128, o384]
                        qp = ps_W.tile([128, C], F32, tag="w")
                        nc.tensor.matmul(
                            qp, lhsT=_f32r(qy1[:, nsl1]), rhs=_f32r(wq1),
                            start=True, stop=False)
                        nc.tensor.matmul(
                            qp, lhsT=_f32r(qy2[:, nsl1]), rhs=_f32r(wq2),
                            start=False, stop=True)
                        qt_sb = ktqp.tile([128, C], F32R, tag="qt")
                        nc.scalar.copy(out=qt_sb, in_=qp)
                        qsq = sqp.tile([128, C], F32R, tag="qsq")
                        nc.vector.tensor_mul(qsq, qt_sb.bitcast(F32), qt_sb.bitcast(F32))
                        nc.tensor.matmul(
                            qn2_ps, lhsT=_f32r(ones[:, 0:1]), rhs=_f32r(qsq),
                            start=(t == 0), stop=(t == nch - 1))
                        # S^T accumulation: [d128, c384] per d-chunk
                        for m in range(3):
                            nc.tensor.matmul(
                                s_ps[m], lhsT=_f32r(kt_sb[:, 128 * m:128 * (m + 1)]),
                                rhs=_f32r(qt_sb),
                                start=(t == 0), stop=(t == nch - 1))
                    # v GEMM for this superchunk: [o128, n512] x 3
                    for mo in range(3):
                        vp = ps_W.tile([128, 512], F32, tag="w")
                        for kc in range(3):
                            nc.tensor.matmul(
                                vp, lhsT=_f32r(wv[:, kc, 128 * mo:128 * (mo + 1)]),
                                rhs=_f32r(xs[:, kc, :]),
                                start=(kc == 0), stop=(kc == 2))
                        nc.vector.tensor_copy(out=v_sb[mo][:, nsl5], in_=vp)

                # ================= phase 2: norms, exp, softmax, av, proj ====
                # 1/max(|k|, eps) and temp/max(|q|, eps) as [1, C] rows
                kn2 = spool.tile([1, C], F32, tag="kn2r")
                qn2 = spool.tile([1, C], F32, tag="qn2r")
                nc.vector.tensor_copy(out=kn2, in_=kn2_ps)
                nc.vector.tensor_copy(out=qn2, in_=qn2_ps)
                nc.vector.tensor_scalar_max(out=kn2, in0=kn2, scalar1=1e-24)
                nc.vector.tensor_scalar_max(out=qn2, in0=qn2, scalar1=1e-24)
                kinvf = spool.tile([1, C], F32, tag="kinvf")
                qinvf = spool.tile([1, C], F32, tag="qinvf")
                nc.scalar.activation(out=kinvf, in_=kn2, func=AF.Sqrt)
                nc.scalar.activation(out=qinvf, in_=qn2, func=AF.Sqrt)
                nc.vector.reciprocal(out=kinvf, in_=kinvf)
                nc.vector.reciprocal(out=qinvf, in_=qinvf)
                # one Newton polish step: y' = y*(1.5 - 0.5*n2*y*y); the final
                # multiply writes the f32r tile the tiny matmuls consume.
                kinv = spool.tile([1, C], F32R, tag="kinv")
                qinv = spool.tile([1, C], F32R, tag="qinv")
                for y, yf, n2 in ((kinv, kinvf, kn2), (qinv, qinvf, qn2)):
                    tpol = spool.tile([1, C], F32, tag="tpol")
                    nc.vector.tensor_mul(tpol, yf, yf)
                    nc.vector.tensor_mul(tpol, tpol, n2)
                    nc.vector.tensor_scalar(
                        out=tpol, in0=tpol, scalar1=-0.5, scalar2=1.5,
                        op0=AluOpType.mult, op1=AluOpType.add)
                    nc.vector.tensor_mul(y, yf, tpol)
                # fold temperature into qinv: temp[h(c)]/|q_c|
                nc.vector.tensor_mul(qinv, qinv.bitcast(F32), trow)

                # broadcast qinv row -> [128, C] via ones-matmul
                qtb_ps = ps_W.tile([128, C], F32, tag="w")
                nc.tensor.matmul(
                    qtb_ps, lhsT=_f32r(ones[0:1, :]), rhs=_f32r(qinv),
                    start=True, stop=True)
                qtb = spool.tile([128, C], F32, tag="qtb")
                nc.vector.tensor_copy(out=qtb, in_=qtb_ps)
                # repartition kinv row -> [128, 1] columns via tiny matmuls
                kcol = spool.tile([128, 3], F32, tag="kcol")
                for m in range(3):
                    kc_ps = ps_W.tile([128, 128], F32, tag="w")
                    nc.tensor.matmul(
                        kc_ps, lhsT=_f32r(kinv[0:1, 128 * m:128 * (m + 1)]),
                        rhs=_f32r(ones[0:1, :]), start=True, stop=True)
                    nc.vector.tensor_copy(out=kcol[:, m:m + 1], in_=kc_ps[:, 0:1])

                # expST[d,c] = maskA * exp(S^T * kinv[d] * qtb[c])
                expst = [epool.tile([128, C], F32R, tag=f"e{m}", name=f"e{m}") for m in range(3)]
                for m in range(3):
                    stt = ktqp.tile([128, C], F32, tag="stt")
                    nc.vector.scalar_tensor_tensor(
                        out=stt, in0=s_ps[m], scalar=kcol[:, m:m + 1], in1=qtb,
                        op0=AluOpType.mult, op1=AluOpType.mult)
                    nc.scalar.activation(out=expst[m], in_=stt, func=AF.Exp)
                    nc.vector.tensor_mul(expst[m], expst[m].bitcast(F32), ma[:, m, :])

                # denominators via the ones-columns of v
                rs = spool.tile([128, 3], F32, tag="rs")
                for m in range(3):
                    dn_ps = ps_W.tile([128, VPAD], F32, tag="w")
                    ks = NZ_BLOCKS[m]
                    for ki, k in enumerate(ks):
                        nc.tensor.matmul(
                            dn_ps, lhsT=_f32r(expst[k][:, 128 * m:128 * (m + 1)]),
                            rhs=_f32r(v_sb[k][:, ntot:ntot + VPAD]),
                            start=(ki == 0), stop=(ki == len(ks) - 1))
                    nc.vector.reciprocal(out=rs[:, m:m + 1], in_=dn_ps[:, 0:1])

                # av + proj, streamed per 512-superchunk
                for ns in range(nsup):
                    nsl5 = slice(ns * 512, (ns + 1) * 512)
                    avs = []
                    for m in range(3):
                        av_ps = ps_W.tile([128, 512], F32, tag="w")
                        ks = NZ_BLOCKS[m]
                        for ki, k in enumerate(ks):
                            nc.tensor.matmul(
                                av_ps, lhsT=_f32r(expst[k][:, 128 * m:128 * (m + 1)]),
                                rhs=_f32r(v_sb[k][:, nsl5]),
                                start=(ki == 0), stop=(ki == len(ks) - 1))
                        av_sb = avsp.tile([128, 512], F32R, tag=f"a{m}")
                        nc.scalar.activation(
                            out=av_sb, in_=av_ps, func=AF.Copy, scale=rs[:, m:m + 1])
                        avs.append(av_sb)
                    for mo in range(3):
                        pp = ps_W.tile([128, 512], F32, tag="w")
                        for kc in range(3):
                            nc.tensor.matmul(
                                pp, lhsT=_f32r(wp[:, kc, 128 * mo:128 * (mo + 1)]),
                                rhs=_f32r(avs[kc]),
                                start=(kc == 0), stop=(kc == 2))
                        po = outp.tile([128, 512], F32, tag="po")
                        nc.scalar.copy(out=po, in_=pp)
                        nc.sync.dma_start(
                            out=out_d[b, 128 * mo:128 * (mo + 1), nsl5], in_=po)
    if finalize:
        nc.finalize()
    return nc


def _host_inputs(x, query, w_kv, w_q, w_proj, temperature):
    """Build the per-core input maps (host-side shard + weight prep)."""
    x = np.ascontiguousarray(x, dtype=np.float32).reshape(B, C, NTOT)
    query = np.ascontiguousarray(query, dtype=np.float32).reshape(B, QC, NTOT)
    wkT = np.ascontiguousarray(w_kv[:C, :].T, dtype=np.float32)
    wvT = np.ascontiguousarray(w_kv[C:, :].T, dtype=np.float32)
    wqT = np.ascontiguousarray(w_q.T, dtype=np.float32)
    wpT = np.ascontiguousarray(w_proj.T, dtype=np.float32)
    maskA = np.kron(np.eye(NUM_HEADS, dtype=np.float32),
                    np.ones((DC, DC), dtype=np.float32))
    trow = np.ascontiguousarray(
        np.repeat(np.asarray(temperature, dtype=np.float32).reshape(NUM_HEADS), DC)
    ).reshape(1, C)
    in_maps = []
    for core in range(NCORES):
        bsl = slice(core * BLOC, (core + 1) * BLOC)
        in_maps.append({
            "x": np.ascontiguousarray(x[bsl]),
            "query": np.ascontiguousarray(query[bsl]),
            "wkT": wkT, "wvT": wvT, "wqT": wqT, "wpT": wpT,
            "maskA": maskA, "trow": trow,
        })
    return in_maps


_NC_CACHE = {}


def _get_nc():
    if "nc" not in _NC_CACHE:
        _NC_CACHE["nc"] = build_bass()
    return _NC_CACHE["nc"]


def kernel(x, query, w_kv, w_q, w_proj, temperature, _trace=False, **trace_kwargs):
    in_maps = _host_inputs(x, query, w_kv, w_q, w_proj, temperature)
    nc = _get_nc()
    res = run_bass_kernel_spmd(
        nc, in_maps, core_ids=list(range(NCORES)), trace=_trace, **trace_kwargs)
    outs = [np.asarray(res.results[i]["out"]) for i in range(NCORES)]
    full = np.concatenate(outs, axis=0).reshape(B, C, H, W).astype(np.float32)
    if _trace:
        return full, res
    return full


# revision 11
# speedup vs baseline: 1.0230x; 1.0230x over previous
"""Trainium2 Bass kernel for nn_CrossAttention (XCA-style channel attention).

Full-input contract: kernel(**inputs) takes the complete tensors, shards the
batch over 8 NeuronCores (2 batches per core, pure data parallel, no
collectives), runs one SPMD Bass program, and reassembles the full output.

Per-core math (per batch):
  kT[n,o] = sum_c x[c,n]   wkT[c,o]      (transposed layout: n on partitions)
  qT[n,o] = sum_c qry[c,n] wqT[c,o]
  v[o,n]  = sum_c wvT[c,o] x[c,n]        (natural layout)
  S^T[d,c] = sum_n kT[n,d] qT[n,c]       (full-cross; only head-diag blocks used)
  norms: |k_d|^2, |q_c|^2 via squares + ones-matmul partition reduction
  expST = maskA * exp(S^T * (1/|k_d|) * (temp[h(c)]/|q_c|))   (zeros off-diag)
  denom[c] = sum_d expST[d,c]            (via ones-columns appended to v)
  av[c,n] = (1/denom[c]) * sum_d expST[d,c] v[d,n]
  out[o,n] = sum_c wpT[c,o] av[c,n]
Softmax needs no max-subtraction: scores are cosine similarities * temperature.

GEMM dtype is switchable: bf16 streams 1 cycle/row through the PE array
(78.6 TF/s); 4-byte fp32r streams at ~2 cycles/row. PSUM accumulation is
fp32 either way; the softmax/normalization pipeline stays fp32.
"""

import os
import sys

import numpy as np

for p in ("/opt/trn_rl_repo", os.path.expanduser("~/.axon_site/_ro/trn_rl_repo")):
    if os.path.isdir(p) and p not in sys.path:
        sys.path.insert(0, p)

import ml_dtypes
import concourse.bass as bass
from concourse import bacc
import concourse.mybir as mybir
import concourse.tile as tile
from concourse.alu_op_type import AluOpType
from concourse.bass_utils import run_bass_kernel_spmd

NUM_HEADS = 8
B, C, QC, H, W = 16, 384, 192, 64, 64
NTOT = H * W          # 4096 spatial positions
DC = C // NUM_HEADS   # 48 channels per head
NCORES = 8
BLOC = B // NCORES    # 2 batches per core
VPAD = 8              # extra ones-columns appended to v for the denominator

F32 = mybir.dt.float32
F32R = mybir.dt.float32r
BF16 = mybir.dt.bfloat16
AF = mybir.ActivationFunctionType

GEMM_BF16 = True  # False -> fp32r GEMMs (half speed, ~10x lower error)

# chunk-pair (d-chunk k, c-chunk m) blocks of the 128-partition-tiled [384,384]
# head-block-diagonal matrix that are not identically zero (heads are 48 wide)
NZ_BLOCKS = {0: [0, 1], 1: [0, 1, 2], 2: [1, 2]}


def build_bass(ntot=NTOT, bloc=BLOC, finalize=True, gemm_bf16=None):
    if gemm_bf16 is None:
        gemm_bf16 = GEMM_BF16
    GDT = BF16 if gemm_bf16 else F32R
    DDT = BF16 if gemm_bf16 else F32  # dram dtype for gemm-fed inputs

    def g(ap):
        # dram-side view for DMA into a GEMM-typed tile
        return ap if gemm_bf16 else ap.bitcast(F32R)

    nc = bacc.Bacc(None)
    nsup = ntot // 512
    nch_per_sup = 4  # 128-chunks per 512 superchunk
    nch = nsup * nch_per_sup

    x_d = nc.declare_dram_parameter("x", [bloc, C, ntot], DDT, isOutput=False)
    qry_d = nc.declare_dram_parameter("query", [bloc, QC, ntot], DDT, isOutput=False)
    wk_d = nc.declare_dram_parameter("wkT", [C, C], DDT, isOutput=False)
    wv_d = nc.declare_dram_parameter("wvT", [C, C], DDT, isOutput=False)
    wq_d = nc.declare_dram_parameter("wqT", [QC, C], DDT, isOutput=False)
    wp_d = nc.declare_dram_parameter("wpT", [C, C], DDT, isOutput=False)
    ma_d = nc.declare_dram_parameter("maskA", [C, C], F32, isOutput=False)
    tr_d = nc.declare_dram_parameter("trow", [1, C], F32, isOutput=False)
    out_d = nc.declare_dram_parameter("out", [bloc, C, ntot], F32, isOutput=True)

    with tile.TileContext(nc) as tc:
        with (
            tc.tile_pool(name="weights", bufs=1) as wpool,
            tc.tile_pool(name="xq", bufs=2) as xpool,
            tc.tile_pool(name="ktq", bufs=3) as ktqp,
            tc.tile_pool(name="sq", bufs=2) as sqp,
            tc.tile_pool(name="v", bufs=1) as vpool,
            tc.tile_pool(name="expst", bufs=1) as epool,
            tc.tile_pool(name="small", bufs=2) as spool,
            tc.tile_pool(name="avs", bufs=2) as avsp,
            tc.tile_pool(name="outs", bufs=3) as outp,
            tc.tile_pool(name="psS", bufs=1, space="PSUM") as ps_S,
            tc.tile_pool(name="psN", bufs=1, space="PSUM") as ps_N,
            tc.tile_pool(name="psW", bufs=3, space="PSUM") as ps_W,
        ):
            # ---- constants ----
            wk = wpool.tile([128, 3, C], GDT, tag="wk")
            wv = wpool.tile([128, 3, C], GDT, tag="wv")
            wp = wpool.tile([128, 3, C], GDT, tag="wp")
            ma = wpool.tile([128, 3, C], F32, tag="ma")
            for kc in range(3):
                nc.sync.dma_start(out=wk[:, kc, :], in_=g(wk_d[128 * kc:128 * (kc + 1), :]))
                nc.sync.dma_start(out=wv[:, kc, :], in_=g(wv_d[128 * kc:128 * (kc + 1), :]))
                nc.sync.dma_start(out=wp[:, kc, :], in_=g(wp_d[128 * kc:128 * (kc + 1), :]))
                nc.sync.dma_start(out=ma[:, kc, :], in_=ma_d[128 * kc:128 * (kc + 1), :])
            wq1 = wpool.tile([128, C], GDT, tag="wq1")
            wq2 = wpool.tile([64, C], GDT, tag="wq2")
            nc.sync.dma_start(out=wq1, in_=g(wq_d[0:128, :]))
            nc.sync.dma_start(out=wq2, in_=g(wq_d[128:192, :]))
            trow = wpool.tile([1, C], F32, tag="trow")
            nc.sync.dma_start(out=trow, in_=tr_d[:, :])
            onesf = wpool.tile([128, 128], F32, tag="onesf")
            nc.vector.memset(onesf, 1.0)
            ones = wpool.tile([128, 128], GDT, tag="ones")
            nc.vector.tensor_copy(out=ones, in_=onesf)
            onesr = wpool.tile([1, 128], F32R, tag="onesr")
            nc.vector.tensor_copy(out=onesr, in_=onesf[0:1, :])

            for b in range(bloc):
                # ---- per-batch persistent tiles ----
                v_sb = [vpool.tile([128, ntot + VPAD], GDT, tag=f"v{k}", name=f"v{k}")
                        for k in range(3)]
                for k in range(3):
                    nc.vector.tensor_copy(
                        out=v_sb[k][:, ntot:ntot + VPAD], in_=onesf[:, 0:VPAD])
                s_ps = [ps_S.tile([128, C], F32, tag=f"S{m}", name=f"S{m}")
                        for m in range(3)]
                kn2_ps = ps_N.tile([1, C], F32, tag="kn2")
                qn2_ps = ps_N.tile([1, C], F32, tag="qn2")

                # ================= phase 1: kT, qT, v, S^T, norm sums ========
                for ns in range(nsup):
                    nsl5 = slice(ns * 512, (ns + 1) * 512)
                    xs = xpool.tile([128, 3, 512], GDT, tag="xs")
                    for kc in range(3):
                        nc.sync.dma_start(
                            out=xs[:, kc, :], in_=g(x_d[b, 128 * kc:128 * (kc + 1), nsl5]))
                    qy1 = xpool.tile([128, 512], GDT, tag="qy1")
                    qy2 = xpool.tile([64, 512], GDT, tag="qy2")
                    nc.sync.dma_start(out=qy1, in_=g(qry_d[b, 0:128, nsl5]))
                    nc.sync.dma_start(out=qy2, in_=g(qry_d[b, 128:192, nsl5]))

                    for nn in range(nch_per_sup):
                        t = ns * nch_per_sup + nn
                        nsl1 = slice(nn * 128, (nn + 1) * 128)
                        # kT chunk: [n128, o384]
                        kp = ps_W.tile([128, C], F32, tag="w")
                        for kc in range(3):
                            nc.tensor.matmul(
                                kp, lhsT=xs[:, kc, nsl1], rhs=wk[:, kc, :],
                                start=(kc == 0), stop=(kc == 2))
                        kt_sb = ktqp.tile([128, C], GDT, tag="kt")
                        nc.vector.tensor_copy(out=kt_sb, in_=kp)
                        ksq = sqp.tile([128, C], GDT, tag="ksq")
                        ktr = kt_sb if gemm_bf16 else kt_sb.bitcast(F32)
                        nc.vector.tensor_mul(ksq, ktr, ktr)
                        nc.tensor.matmul(
                            kn2_ps, lhsT=ones[:, 0:1], rhs=ksq,
                            start=(t == 0), stop=(t == nch - 1))
                        # qT chunk: [n128, o384]
                        qp = ps_W.tile([128, C], F32, tag="w")
                        nc.tensor.matmul(
                            qp, lhsT=qy1[:, nsl1], rhs=wq1, start=True, stop=False)
                        nc.tensor.matmul(
                            qp, lhsT=qy2[:, nsl1], rhs=wq2, start=False, stop=True)
                        qt_sb = ktqp.tile([128, C], GDT, tag="qt")
                        nc.scalar.copy(out=qt_sb, in_=qp)
                        qsq = sqp.tile([128, C], GDT, tag="qsq")
                        qtr = qt_sb if gemm_bf16 else qt_sb.bitcast(F32)
                        nc.vector.tensor_mul(qsq, qtr, qtr)
                        nc.tensor.matmul(
                            qn2_ps, lhsT=ones[:, 0:1], rhs=qsq,
                            start=(t == 0), stop=(t == nch - 1))
                        # S^T accumulation: [d128, c384] per d-chunk
                        for m in range(3):
                            nc.tensor.matmul(
                                s_ps[m], lhsT=kt_sb[:, 128 * m:128 * (m + 1)],
                                rhs=qt_sb, start=(t == 0), stop=(t == nch - 1))
                    # v GEMM for this superchunk: [o128, n512] x 3
                    for mo in range(3):
                        vp = ps_W.tile([128, 512], F32, tag="w")
                        for kc in range(3):
                            nc.tensor.matmul(
                                vp, lhsT=wv[:, kc, 128 * mo:128 * (mo + 1)],
                                rhs=xs[:, kc, :], start=(kc == 0), stop=(kc == 2))
                        nc.vector.tensor_copy(out=v_sb[mo][:, nsl5], in_=vp)

                # ================= phase 2: norms, exp, softmax, av, proj ====
                # 1/max(|k|,eps), temp/max(|q|,eps) as [1, C] fp32 rows
                kn2 = spool.tile([1, C], F32, tag="kn2r")
                qn2 = spool.tile([1, C], F32, tag="qn2r")
                nc.vector.tensor_copy(out=kn2, in_=kn2_ps)
                nc.vector.tensor_copy(out=qn2, in_=qn2_ps)
                nc.vector.tensor_scalar_max(out=kn2, in0=kn2, scalar1=1e-24)
                nc.vector.tensor_scalar_max(out=qn2, in0=qn2, scalar1=1e-24)
                kinvf = spool.tile([1, C], F32, tag="kinvf")
                qinvf = spool.tile([1, C], F32, tag="qinvf")
                nc.scalar.activation(out=kinvf, in_=kn2, func=AF.Sqrt)
                nc.scalar.activation(out=qinvf, in_=qn2, func=AF.Sqrt)
                nc.vector.reciprocal(out=kinvf, in_=kinvf)
                nc.vector.reciprocal(out=qinvf, in_=qinvf)
                # one Newton polish step: y' = y*(1.5 - 0.5*n2*y*y); final
                # multiply writes the f32r row the tiny matmuls consume.
                kinv = spool.tile([1, C], F32R, tag="kinv")
                qinv = spool.tile([1, C], F32R, tag="qinv")
                for y, yf, n2 in ((kinv, kinvf, kn2), (qinv, qinvf, qn2)):
                    tpol = spool.tile([1, C], F32, tag="tpol")
                    nc.vector.tensor_mul(tpol, yf, yf)
                    nc.vector.tensor_mul(tpol, tpol, n2)
                    nc.vector.tensor_scalar(
                        out=tpol, in0=tpol, scalar1=-0.5, scalar2=1.5,
                        op0=AluOpType.mult, op1=AluOpType.add)
                    nc.vector.tensor_mul(y, yf, tpol)
                # fold temperature into qinv: temp[h(c)]/|q_c|
                nc.vector.tensor_mul(qinv, qinv.bitcast(F32), trow)

                # broadcast qinv row -> [128, C] via ones-matmul (K=1)
                qtb_ps = ps_W.tile([128, C], F32, tag="w")
                nc.tensor.matmul(
                    qtb_ps, lhsT=onesr, rhs=qinv, start=True, stop=True)
                qtb = spool.tile([128, C], F32, tag="qtb")
                nc.vector.tensor_copy(out=qtb, in_=qtb_ps)
                # repartition kinv row -> [128,1] columns via K=1 matmuls
                kcol = spool.tile([128, 3], F32, tag="kcol")
                for m in range(3):
                    kc_ps = ps_W.tile([128, 128], F32, tag="w")
                    nc.tensor.matmul(
                        kc_ps, lhsT=kinv[0:1, 128 * m:128 * (m + 1)],
                        rhs=onesr, start=True, stop=True)
                    nc.vector.tensor_copy(out=kcol[:, m:m + 1], in_=kc_ps[:, 0:1])

                # expST[d,c] = maskA * exp(S^T * kinv[d] * qtb[c])
                expst = [epool.tile([128, C], GDT, tag=f"e{m}", name=f"e{m}")
                         for m in range(3)]
                for m in range(3):
                    stt = ktqp.tile([128, C], F32, tag="stt")
                    nc.vector.scalar_tensor_tensor(
                        out=stt, in0=s_ps[m], scalar=kcol[:, m:m + 1], in1=qtb,
                        op0=AluOpType.mult, op1=AluOpType.mult)
                    ex = ktqp.tile([128, C], F32, tag="ex")
                    nc.scalar.activation(out=ex, in_=stt, func=AF.Exp)
                    nc.vector.tensor_mul(expst[m], ex, ma[:, m, :])

                # denominators via the ones-columns of v
                rs = spool.tile([128, 3], F32, tag="rs")
                for m in range(3):
                    dn_ps = ps_W.tile([128, VPAD], F32, tag="w")
                    ks = NZ_BLOCKS[m]
                    for ki, k in enumerate(ks):
                        nc.tensor.matmul(
                            dn_ps, lhsT=expst[k][:, 128 * m:128 * (m + 1)],
                            rhs=v_sb[k][:, ntot:ntot + VPAD],
                            start=(ki == 0), stop=(ki == len(ks) - 1))
                    nc.vector.reciprocal(out=rs[:, m:m + 1], in_=dn_ps[:, 0:1])

                # av + proj, streamed per 512-superchunk
                for ns in range(nsup):
                    nsl5 = slice(ns * 512, (ns + 1) * 512)
                    avs = []
                    for m in range(3):
                        av_ps = ps_W.tile([128, 512], F32, tag="w")
                        ks = NZ_BLOCKS[m]
                        for ki, k in enumerate(ks):
                            nc.tensor.matmul(
                                av_ps, lhsT=expst[k][:, 128 * m:128 * (m + 1)],
                                rhs=v_sb[k][:, nsl5],
                                start=(ki == 0), stop=(ki == len(ks) - 1))
                        av_sb = avsp.tile([128, 512], GDT, tag=f"a{m}", name=f"a{m}")
                        nc.scalar.activation(
                            out=av_sb, in_=av_ps, func=AF.Copy, scale=rs[:, m:m + 1])
                        avs.append(av_sb)
                    for mo in range(3):
                        pp = ps_W.tile([128, 512], F32, tag="w")
                        for kc in range(3):
                            nc.tensor.matmul(
                                pp, lhsT=wp[:, kc, 128 * mo:128 * (mo + 1)],
                                rhs=avs[kc], start=(kc == 0), stop=(kc == 2))
                        po = outp.tile([128, 512], F32, tag="po")
                        nc.scalar.copy(out=po, in_=pp)
                        nc.sync.dma_start(
                            out=out_d[b, 128 * mo:128 * (mo + 1), nsl5], in_=po)
    if finalize:
        nc.finalize()
    return nc


def _host_inputs(x, query, w_kv, w_q, w_proj, temperature, gemm_bf16=None):
    """Build the per-core input maps (host-side shard + weight prep)."""
    if gemm_bf16 is None:
        gemm_bf16 = GEMM_BF16
    dt = ml_dtypes.bfloat16 if gemm_bf16 else np.float32
    x = np.ascontiguousarray(np.asarray(x, dtype=np.float32).reshape(B, C, NTOT), dtype=dt)
    query = np.ascontiguousarray(
        np.asarray(query, dtype=np.float32).reshape(B, QC, NTOT), dtype=dt)
    wkT = np.ascontiguousarray(np.asarray(w_kv[:C, :]).T, dtype=dt)
    wvT = np.ascontiguousarray(np.asarray(w_kv[C:, :]).T, dtype=dt)
    wqT = np.ascontiguousarray(np.asarray(w_q).T, dtype=dt)
    wpT = np.ascontiguousarray(np.asarray(w_proj).T, dtype=dt)
    maskA = np.kron(np.eye(NUM_HEADS, dtype=np.float32),
                    np.ones((DC, DC), dtype=np.float32))
    trow = np.ascontiguousarray(
        np.repeat(np.asarray(temperature, dtype=np.float32).reshape(NUM_HEADS), DC)
    ).reshape(1, C)
    in_maps = []
    for core in range(NCORES):
        bsl = slice(core * BLOC, (core + 1) * BLOC)
        in_maps.append({
            "x": np.ascontiguousarray(x[bsl]),
            "query": np.ascontiguousarray(query[bsl]),
            "wkT": wkT, "wvT": wvT, "wqT": wqT, "wpT": wpT,
            "maskA": maskA, "trow": trow,
        })
    return in_maps


_NC_CACHE = {}


def _get_nc():
    if "nc" not in _NC_CACHE:
        _NC_CACHE["nc"] = build_bass()
    return _NC_CACHE["nc"]


def kernel(x, query, w_kv, w_q, w_proj, temperature, _trace=False, **trace_kwargs):
    in_maps = _host_inputs(x, query, w_kv, w_q, w_proj, temperature)
    nc = _get_nc()
    res = run_bass_kernel_spmd(
        nc, in_maps, core_ids=list(range(NCORES)), trace=_trace, **trace_kwargs)
    outs = [np.asarray(res.results[i]["out"]) for i in range(NCORES)]
    full = np.concatenate(outs, axis=0).reshape(B, C, H, W).astype(np.float32)
    if _trace:
        return full, res
    return full


# revision 14
# speedup vs baseline: 1.0329x; 1.0096x over previous
"""Trainium2 Bass kernel for nn_CrossAttention (XCA-style channel attention).

Full-input contract: kernel(**inputs) takes the complete tensors, shards the
batch over 8 NeuronCores (2 batches per core, pure data parallel, no
collectives), runs one SPMD Bass program, and reassembles the full output.

Per-core math (per batch):
  kT[n,o] = sum_c x[c,n]   wkT[c,o]      (transposed layout: n on partitions)
  qT[n,o] = sum_c qry[c,n] wqT[c,o]
  v[o,n]  = sum_c wvT[c,o] x[c,n]        (natural layout)
  S^T[d,c] = sum_n kT[n,d] qT[n,c]       (full-cross; only head-diag blocks used)
  norms: |k_d|^2, |q_c|^2 via squares + ones-matmul partition reduction
  expST = maskA * exp(S^T * (1/|k_d|) * (temp[h(c)]/|q_c|))   (zeros off-diag)
  denom[c] = sum_d expST[d,c]            (via ones-columns appended to v)
  av[c,n] = (1/denom[c]) * sum_d expST[d,c] v[d,n]
  out[o,n] = sum_c wpT[c,o] av[c,n]
Softmax needs no max-subtraction: scores are cosine similarities * temperature.

The two local batches are software-pipelined: both phase-1 GEMM passes are
emitted before either softmax/av/proj phase, so the PE fills batch-boundary
stalls (norm pipeline latency) with the other batch's GEMMs.

GEMM dtype is switchable: bf16 streams 1 cycle/row through the PE array
(78.6 TF/s); 4-byte fp32r streams at ~2 cycles/row. PSUM accumulation is
fp32 either way; the softmax/normalization pipeline stays fp32.
"""

import os
import sys

import numpy as np

for p in ("/opt/trn_rl_repo", os.path.expanduser("~/.axon_site/_ro/trn_rl_repo")):
    if os.path.isdir(p) and p not in sys.path:
        sys.path.insert(0, p)

import ml_dtypes
import concourse.bass as bass
from concourse import bacc
import concourse.mybir as mybir
import concourse.tile as tile
from concourse.alu_op_type import AluOpType
from concourse.bass_utils import run_bass_kernel_spmd

NUM_HEADS = 8
B, C, QC, H, W = 16, 384, 192, 64, 64
NTOT = H * W          # 4096 spatial positions
DC = C // NUM_HEADS   # 48 channels per head
NCORES = 8
BLOC = B // NCORES    # 2 batches per core
VPAD = 8              # extra ones-columns appended to v for the denominator

F32 = mybir.dt.float32
F32R = mybir.dt.float32r
BF16 = mybir.dt.bfloat16
AF = mybir.ActivationFunctionType

GEMM_BF16 = True  # False -> fp32r GEMMs (half speed, ~10x lower error)

# chunk-pair (d-chunk k, c-chunk m) blocks of the 128-partition-tiled [384,384]
# head-block-diagonal matrix that are not identically zero (heads are 48 wide)
NZ_BLOCKS = {0: [0, 1], 1: [0, 1, 2], 2: [1, 2]}


def build_bass(ntot=NTOT, bloc=BLOC, finalize=True, gemm_bf16=None):
    if gemm_bf16 is None:
        gemm_bf16 = GEMM_BF16
    GDT = BF16 if gemm_bf16 else F32R
    DDT = BF16 if gemm_bf16 else F32  # dram dtype for gemm-fed inputs

    def g(ap):
        # dram-side view for DMA into a GEMM-typed tile
        return ap if gemm_bf16 else ap.bitcast(F32R)

    nc = bacc.Bacc(None)
    nsup = ntot // 512
    nch_per_sup = 4  # 128-chunks per 512 superchunk
    nch = nsup * nch_per_sup

    x_d = nc.declare_dram_parameter("x", [bloc, C, ntot], DDT, isOutput=False)
    qry_d = nc.declare_dram_parameter("query", [bloc, QC, ntot], DDT, isOutput=False)
    wk_d = nc.declare_dram_parameter("wkT", [C, C], DDT, isOutput=False)
    wv_d = nc.declare_dram_parameter("wvT", [C, C], DDT, isOutput=False)
    wq_d = nc.declare_dram_parameter("wqT", [QC, C], DDT, isOutput=False)
    wp_d = nc.declare_dram_parameter("wpT", [C, C], DDT, isOutput=False)
    ma_d = nc.declare_dram_parameter("maskA", [C, C], F32, isOutput=False)
    tr_d = nc.declare_dram_parameter("trow", [1, C], F32, isOutput=False)
    out_d = nc.declare_dram_parameter("out", [bloc, C, ntot], F32, isOutput=True)

    with tile.TileContext(nc) as tc:
        with (
            tc.tile_pool(name="weights", bufs=1) as wpool,
            tc.tile_pool(name="xq", bufs=2) as xpool,
            tc.tile_pool(name="ktq", bufs=8) as ktqp,
            tc.tile_pool(name="sq", bufs=3) as sqp,
            tc.tile_pool(name="stt", bufs=2) as sttp,
            tc.tile_pool(name="v", bufs=2) as vpool,
            tc.tile_pool(name="expst", bufs=2) as epool,
            tc.tile_pool(name="small", bufs=2) as spool,
            tc.tile_pool(name="avs", bufs=2) as avsp,
            tc.tile_pool(name="outs", bufs=4) as outp,
            tc.tile_pool(name="psS", bufs=1, space="PSUM") as ps_S,
            tc.tile_pool(name="psN", bufs=1, space="PSUM") as ps_N,
            tc.tile_pool(name="psW", bufs=2, space="PSUM") as ps_W,
            tc.tile_pool(name="psX", bufs=1, space="PSUM") as ps_X,
        ):
            # ---- constants ----
            wk = wpool.tile([128, 3, C], GDT, tag="wk")
            wv = wpool.tile([128, 3, C], GDT, tag="wv")
            wp = wpool.tile([128, 3, C], GDT, tag="wp")
            ma = wpool.tile([128, 3, C], F32, tag="ma")
            for kc in range(3):
                nc.sync.dma_start(out=wk[:, kc, :], in_=g(wk_d[128 * kc:128 * (kc + 1), :]))
                nc.sync.dma_start(out=wv[:, kc, :], in_=g(wv_d[128 * kc:128 * (kc + 1), :]))
                nc.sync.dma_start(out=wp[:, kc, :], in_=g(wp_d[128 * kc:128 * (kc + 1), :]))
                nc.sync.dma_start(out=ma[:, kc, :], in_=ma_d[128 * kc:128 * (kc + 1), :])
            wq1 = wpool.tile([128, C], GDT, tag="wq1")
            wq2 = wpool.tile([64, C], GDT, tag="wq2")
            nc.sync.dma_start(out=wq1, in_=g(wq_d[0:128, :]))
            nc.sync.dma_start(out=wq2, in_=g(wq_d[128:192, :]))
            trow = wpool.tile([1, C], F32, tag="trow")
            nc.sync.dma_start(out=trow, in_=tr_d[:, :])
            onesf = wpool.tile([128, 128], F32, tag="onesf")
            nc.vector.memset(onesf, 1.0)
            ones = wpool.tile([128, 128], GDT, tag="ones")
            nc.vector.tensor_copy(out=ones, in_=onesf)
            onesr = wpool.tile([1, 128], F32R, tag="onesr")
            nc.vector.tensor_copy(out=onesr, in_=onesf[0:1, :])

            st = [dict() for _ in range(bloc)]  # per-batch cross-phase state

            def phase1(b):
                s = st[b]
                v_sb = s["v"] = [
                    vpool.tile([128, ntot + VPAD], GDT, tag=f"v{k}", name=f"vt{k}")
                    for k in range(3)]
                for k in range(3):
                    nc.vector.tensor_copy(
                        out=v_sb[k][:, ntot:ntot + VPAD], in_=onesf[:, 0:VPAD])
                s_ps = s["S"] = [
                    ps_S.tile([128, C], F32, tag=f"S{m}", name=f"St{m}")
                    for m in range(3)]
                kn2_ps = s["kn2"] = ps_N.tile([1, C], F32, tag="kn2", name="kn2p")
                qn2_ps = s["qn2"] = ps_N.tile([1, C], F32, tag="qn2", name="qn2p")

                for ns in range(nsup):
                    nsl5 = slice(ns * 512, (ns + 1) * 512)
                    xs = xpool.tile([128, 3, 512], GDT, tag="xs")
                    for kc in range(3):
                        nc.sync.dma_start(
                            out=xs[:, kc, :], in_=g(x_d[b, 128 * kc:128 * (kc + 1), nsl5]))
                    qy1 = xpool.tile([128, 512], GDT, tag="qy1")
                    qy2 = xpool.tile([64, 512], GDT, tag="qy2")
                    nc.sync.dma_start(out=qy1, in_=g(qry_d[b, 0:128, nsl5]))
                    nc.sync.dma_start(out=qy2, in_=g(qry_d[b, 128:192, nsl5]))

                    for nn in range(nch_per_sup):
                        t = ns * nch_per_sup + nn
                        nsl1 = slice(nn * 128, (nn + 1) * 128)
                        kp = ps_W.tile([128, C], F32, tag="w")
                        for kc in range(3):
                            nc.tensor.matmul(
                                kp, lhsT=xs[:, kc, nsl1], rhs=wk[:, kc, :],
                                start=(kc == 0), stop=(kc == 2))
                        kt_sb = ktqp.tile([128, C], GDT, tag="kt")
                        nc.vector.tensor_copy(out=kt_sb, in_=kp)
                        ksq = sqp.tile([128, C], GDT, tag="ksq")
                        ktr = kt_sb if gemm_bf16 else kt_sb.bitcast(F32)
                        nc.vector.tensor_mul(ksq, ktr, ktr)
                        nc.tensor.matmul(
                            kn2_ps, lhsT=ones[:, 0:1], rhs=ksq,
                            start=(t == 0), stop=(t == nch - 1))
                        qp = ps_W.tile([128, C], F32, tag="w")
                        nc.tensor.matmul(
                            qp, lhsT=qy1[:, nsl1], rhs=wq1, start=True, stop=False)
                        nc.tensor.matmul(
                            qp, lhsT=qy2[:, nsl1], rhs=wq2, start=False, stop=True)
                        qt_sb = ktqp.tile([128, C], GDT, tag="qt")
                        nc.scalar.copy(out=qt_sb, in_=qp)
                        qsq = sqp.tile([128, C], GDT, tag="qsq")
                        qtr = qt_sb if gemm_bf16 else qt_sb.bitcast(F32)
                        nc.vector.tensor_mul(qsq, qtr, qtr)
                        nc.tensor.matmul(
                            qn2_ps, lhsT=ones[:, 0:1], rhs=qsq,
                            start=(t == 0), stop=(t == nch - 1))
                        for m in range(3):
                            nc.tensor.matmul(
                                s_ps[m], lhsT=kt_sb[:, 128 * m:128 * (m + 1)],
                                rhs=qt_sb, start=(t == 0), stop=(t == nch - 1))
                    for mo in range(3):
                        vp = ps_W.tile([128, 512], F32, tag="w")
                        for kc in range(3):
                            nc.tensor.matmul(
                                vp, lhsT=wv[:, kc, 128 * mo:128 * (mo + 1)],
                                rhs=xs[:, kc, :], start=(kc == 0), stop=(kc == 2))
                        nc.vector.tensor_copy(out=v_sb[mo][:, nsl5], in_=vp)

            def norms(b):
                """Norm sums -> per-chunk scale columns + broadcast row."""
                s = st[b]
                # k side: repartition raw |k|^2 sums into [128, 3] columns via
                # K=1 matmuls, then one wide rsqrt on the columns.
                kn2r = spool.tile([1, C], F32R, tag="kn2r")
                nc.vector.tensor_copy(out=kn2r, in_=s["kn2"])
                n2col = spool.tile([128, 3], F32, tag="n2col")
                for m in range(3):
                    kc_ps = ps_X.tile([128, 128], F32, tag="x")
                    nc.tensor.matmul(
                        kc_ps, lhsT=kn2r[0:1, 128 * m:128 * (m + 1)],
                        rhs=onesr, start=True, stop=True)
                    nc.vector.tensor_copy(out=n2col[:, m:m + 1], in_=kc_ps[:, 0:1])
                nc.vector.tensor_scalar_max(out=n2col, in0=n2col, scalar1=1e-24)
                kinvcol = s["kinvcol"] = spool.tile([128, 3], F32, tag="kinvcol", name="kinvcol")
                nc.scalar.activation(
                    out=kinvcol, in_=n2col, func=AF.Abs_reciprocal_sqrt)
                # q side: rsqrt on the [1, C] row, fold temperature, broadcast
                # to [128, C] via a K=1 ones-matmul.
                qn2 = spool.tile([1, C], F32, tag="qn2r")
                nc.vector.tensor_copy(out=qn2, in_=s["qn2"])
                nc.vector.tensor_scalar_max(out=qn2, in0=qn2, scalar1=1e-24)
                qinvf = spool.tile([1, C], F32, tag="qinvf")
                nc.scalar.activation(out=qinvf, in_=qn2, func=AF.Abs_reciprocal_sqrt)
                qinv = spool.tile([1, C], F32R, tag="qinv")
                nc.vector.tensor_mul(qinv, qinvf, trow)
                qtb_ps = ps_X.tile([128, C], F32, tag="x")
                nc.tensor.matmul(qtb_ps, lhsT=onesr, rhs=qinv, start=True, stop=True)
                qtb = s["qtb"] = spool.tile([128, C], F32, tag="qtb", name="qtb")
                nc.vector.tensor_copy(out=qtb, in_=qtb_ps)

            def phase2(b):
                s = st[b]
                v_sb, s_ps = s["v"], s["S"]
                norms(b)
                kinvcol, qtb = s["kinvcol"], s["qtb"]
                expst = [epool.tile([128, C], GDT, tag=f"e{m}", name=f"et{m}")
                         for m in range(3)]
                for m in range(3):
                    stt = sttp.tile([128, C], F32, tag="stt")
                    nc.vector.scalar_tensor_tensor(
                        out=stt, in0=s_ps[m], scalar=kinvcol[:, m:m + 1], in1=qtb,
                        op0=AluOpType.mult, op1=AluOpType.mult)
                    ex = sttp.tile([128, C], F32, tag="ex")
                    nc.scalar.activation(out=ex, in_=stt, func=AF.Exp)
                    nc.vector.tensor_mul(expst[m], ex, ma[:, m, :])

                rs = spool.tile([128, 3], F32, tag="rs")
                for m in range(3):
                    dn_ps = ps_X.tile([128, VPAD], F32, tag="x")
                    ks = NZ_BLOCKS[m]
                    for ki, k in enumerate(ks):
                        nc.tensor.matmul(
                            dn_ps, lhsT=expst[k][:, 128 * m:128 * (m + 1)],
                            rhs=v_sb[k][:, ntot:ntot + VPAD],
                            start=(ki == 0), stop=(ki == len(ks) - 1))
                    nc.vector.reciprocal(out=rs[:, m:m + 1], in_=dn_ps[:, 0:1])

                for ns in range(nsup):
                    nsl5 = slice(ns * 512, (ns + 1) * 512)
                    avs = []
                    for m in range(3):
                        av_ps = ps_W.tile([128, 512], F32, tag="w")
                        ks = NZ_BLOCKS[m]
                        for ki, k in enumerate(ks):
                            nc.tensor.matmul(
                                av_ps, lhsT=expst[k][:, 128 * m:128 * (m + 1)],
                                rhs=v_sb[k][:, nsl5],
                                start=(ki == 0), stop=(ki == len(ks) - 1))
                        av_sb = avsp.tile([128, 512], GDT, tag=f"a{m}", name=f"at{m}")
                        nc.scalar.activation(
                            out=av_sb, in_=av_ps, func=AF.Copy, scale=rs[:, m:m + 1])
                        avs.append(av_sb)
                    for mo in range(3):
                        pp = ps_W.tile([128, 512], F32, tag="w")
                        for kc in range(3):
                            nc.tensor.matmul(
                                pp, lhsT=wp[:, kc, 128 * mo:128 * (mo + 1)],
                                rhs=avs[kc], start=(kc == 0), stop=(kc == 2))
                        po = outp.tile([128, 512], F32, tag="po")
                        nc.scalar.copy(out=po, in_=pp)
                        nc.sync.dma_start(
                            out=out_d[b, 128 * mo:128 * (mo + 1), nsl5], in_=po)

            # software pipeline: all phase-1 GEMMs first, then the softmax/
            # av/proj phases — the scheduler fills norm-latency stalls of one
            # batch with the other batch's GEMM work.
            for b in range(bloc):
                phase1(b)
            for b in range(bloc):
                phase2(b)
    if finalize:
        nc.finalize()
    return nc


def _host_inputs(x, query, w_kv, w_q, w_proj, temperature, gemm_bf16=None):
    """Build the per-core input maps (host-side shard + weight prep)."""
    if gemm_bf16 is None:
        gemm_bf16 = GEMM_BF16
    dt = ml_dtypes.bfloat16 if gemm_bf16 else np.float32
    x = np.ascontiguousarray(np.asarray(x, dtype=np.float32).reshape(B, C, NTOT), dtype=dt)
    query = np.ascontiguousarray(
        np.asarray(query, dtype=np.float32).reshape(B, QC, NTOT), dtype=dt)
    wkT = np.ascontiguousarray(np.asarray(w_kv[:C, :]).T, dtype=dt)
    wvT = np.ascontiguousarray(np.asarray(w_kv[C:, :]).T, dtype=dt)
    wqT = np.ascontiguousarray(np.asarray(w_q).T, dtype=dt)
    wpT = np.ascontiguousarray(np.asarray(w_proj).T, dtype=dt)
    maskA = np.kron(np.eye(NUM_HEADS, dtype=np.float32),
                    np.ones((DC, DC), dtype=np.float32))
    trow = np.ascontiguousarray(
        np.repeat(np.asarray(temperature, dtype=np.float32).reshape(NUM_HEADS), DC)
    ).reshape(1, C)
    in_maps = []
    for core in range(NCORES):
        bsl = slice(core * BLOC, (core + 1) * BLOC)
        in_maps.append({
            "x": np.ascontiguousarray(x[bsl]),
            "query": np.ascontiguousarray(query[bsl]),
            "wkT": wkT, "wvT": wvT, "wqT": wqT, "wpT": wpT,
            "maskA": maskA, "trow": trow,
        })
    return in_maps


_NC_CACHE = {}


def _get_nc():
    if "nc" not in _NC_CACHE:
        _NC_CACHE["nc"] = build_bass()
    return _NC_CACHE["nc"]


def kernel(x, query, w_kv, w_q, w_proj, temperature, _trace=False, **trace_kwargs):
    in_maps = _host_inputs(x, query, w_kv, w_q, w_proj, temperature)
    nc = _get_nc()
    res = run_bass_kernel_spmd(
        nc, in_maps, core_ids=list(range(NCORES)), trace=_trace, **trace_kwargs)
    outs = [np.asarray(res.results[i]["out"]) for i in range(NCORES)]
    full = np.concatenate(outs, axis=0).reshape(B, C, H, W).astype(np.float32)
    if _trace:
        return full, res
    return full
